# revision 1
# baseline (speedup 1.0000x reference)
"""Trainium2 Bass kernel for the conv-projected self-attention block.

Reference computation (B=8, C=256, N=64, K=256):
    q = wq @ x + bq; k = wk @ x + bk; v = wv @ x + bv      (1x1 convs over C)
    s = einsum('bcnk,bcnl->bnkl', q, k) / sqrt(C)
    p = softmax(s, axis=-1)
    o = einsum('bnkl,bcnl->bcnk', p, v)
    out = x + (wp @ o + bp)

Sharding: data-parallel over B — one batch per NeuronCore (8 cores).
All matmuls run in fp32r (full PE rate at free-dim>=256, ~fp32 accuracy).

Per-core dataflow (batch b), per group of G n-slices:
  load X[ci-tile, G*256]                                   (2 DMAs)
  Q = wqT.T @ X + bq          -> [c, pos]   (ACT Identity+bias evict)
  K = wkT.T @ X + bk          -> [c, pos]   (evicts alternate DVE/ACT)
  Vt = X.T @ wvT  (no bias)   -> [pos, c]   (evicts alternate DVE/ACT;
                                             bv folds into bp' = bp + wp@bv)
  per PAIR of slices (N=512 tails):
    St = K.T @ Q              -> [l, kk] PSUM        (per slice)
    Ep = exp(St/16)           -> SBUF f32r (ACT; scores are bounded so no
                                            max-subtraction is needed)
    Sum = ones.T @ Ep         -> [128, 2*kk] PSUM (sum replicated across
                                 partitions by the all-ones stationary op)
    R = 1/Sum                 (DVE reciprocal_approx_fast, ~18 bits)
    Att = (Vt.T @ Ep) * R     (per slice; DVE tensor_tensor evict with a
                               0-stride broadcast of R over the c-tiles)
    Out = (wpT.T @ Att + bp') + x   (DVE affine_then_add evict)
    store Out                 (2 DMAs per pair, PSUM->SBUF->DRAM)

V is produced directly transposed (X as the stationary operand) so the
whole attention pipeline needs no on-chip transposes.
"""

import numpy as np

import concourse.bass as bass
import concourse.bacc as bacc
import concourse.mybir as mybir
import concourse.tile as tile
from concourse.bass_utils import run_bass_kernel_spmd

F32 = mybir.dt.float32
F32R = mybir.dt.float32r
AF = mybir.ActivationFunctionType
ALU = mybir.AluOpType

B, C, N, K = 8, 256, 64, 256
# group sizes (n-slices per group): small leading groups so the first
# projection matmuls start after a small X transfer, 8-wide steady state
GROUPS = [2, 6] + [8] * 6 + [6, 2]
assert sum(GROUPS) == N
GMAX = max(GROUPS)
SCALE = 1.0 / 16.0    # 1/sqrt(C)

_CACHE = {}


def _build():
    nc = bacc.Bacc("TRN2", target_bir_lowering=False, debug=False, num_devices=8)

    x_d = nc.dram_tensor("x", [C, N, K], F32R, kind="ExternalInput")
    wqt_d = nc.dram_tensor("wqt", [C, C], F32R, kind="ExternalInput")
    wkt_d = nc.dram_tensor("wkt", [C, C], F32R, kind="ExternalInput")
    wvt_d = nc.dram_tensor("wvt", [C, C], F32R, kind="ExternalInput")
    wpt_d = nc.dram_tensor("wpt", [C, C], F32R, kind="ExternalInput")
    bq_d = nc.dram_tensor("bq", [C], F32, kind="ExternalInput")
    bk_d = nc.dram_tensor("bk", [C], F32, kind="ExternalInput")
    bpe_d = nc.dram_tensor("bpe", [C], F32, kind="ExternalInput")
    y_d = nc.dram_tensor("y", [C, N, K], F32, kind="ExternalOutput")

    with tile.TileContext(nc) as tc:
        with tc.tile_pool(name="const", bufs=1) as const, \
             tc.tile_pool(name="xg", bufs=2) as xgp, \
             tc.tile_pool(name="qk", bufs=2) as qkp, \
             tc.tile_pool(name="vt", bufs=2) as vtp, \
             tc.tile_pool(name="sm", bufs=3) as smp, \
             tc.tile_pool(name="ot", bufs=3) as otp, \
             tc.tile_pool(name="ps_proj", bufs=3, space="PSUM") as ps_proj, \
             tc.tile_pool(name="ps_attn", bufs=3, space="PSUM") as ps_attn, \
             tc.tile_pool(name="ps_fin", bufs=2, space="PSUM") as ps_fin:

            # ---- first (small) X group loads before everything else:
            # its transfer gates the very first matmul ----
            xg_first = xgp.tile([128, 2, GROUPS[0], K], F32R, name="xg",
                                tag="xg")
            for ci in range(2):
                nc.sync.dma_start(out=xg_first[:, ci, :, :],
                                  in_=x_d[bass.ts(ci, 128), 0:GROUPS[0], :])

            # ---- constants ----
            wqt = const.tile([128, 2, C], F32R, name="wqt_s")   # dim1 = ci tile
            wkt = const.tile([128, 2, C], F32R, name="wkt_s")
            wvt = const.tile([128, 2, C], F32R, name="wvt_s")
            wpt = const.tile([128, 2, C], F32R, name="wpt_s")
            for ci in range(2):
                nc.sync.dma_start(out=wqt[:, ci, :], in_=wqt_d[bass.ts(ci, 128), :])
                nc.sync.dma_start(out=wkt[:, ci, :], in_=wkt_d[bass.ts(ci, 128), :])
                nc.sync.dma_start(out=wvt[:, ci, :], in_=wvt_d[bass.ts(ci, 128), :])
                nc.sync.dma_start(out=wpt[:, ci, :], in_=wpt_d[bass.ts(ci, 128), :])
            bq_s = const.tile([128, 2], F32, name="bq_s")
            bk_s = const.tile([128, 2], F32, name="bk_s")
            bpe_s = const.tile([128, 2], F32, name="bpe_s")
            for t in range(2):
                nc.sync.dma_start(out=bq_s[:, t:t + 1], in_=bq_d[bass.ts(t, 128)])
                nc.sync.dma_start(out=bk_s[:, t:t + 1], in_=bk_d[bass.ts(t, 128)])
                nc.sync.dma_start(out=bpe_s[:, t:t + 1], in_=bpe_d[bass.ts(t, 128)])
            ones_f = const.tile([128, 256], F32, name="ones_f")
            nc.vector.memset(ones_f, 1.0)
            ones = const.tile([128, 256], F32R, name="ones_s")
            nc.vector.tensor_copy(ones, ones_f)

            n0 = 0
            for g, G in enumerate(GROUPS):
                # ---- load X group: one tile [128, 2(ci), G, 256] ----
                if g == 0:
                    xg = xg_first
                else:
                    xg = xgp.tile([128, 2, G, K], F32R, name="xg", tag="xg")
                    for ci in range(2):
                        nc.sync.dma_start(
                            out=xg[:, ci, :, :],
                            in_=x_d[bass.ts(ci, 128), n0:n0 + G, :])

                # ---- Q/K projections: [128, 2(co), G, 256] ----
                qg = qkp.tile([128, 2, G, K], F32R, name="qg", tag="qg")
                kg = qkp.tile([128, 2, G, K], F32R, name="kg", tag="kg")
                nch = (G * K) // 512  # 512-wide chunks per group
                for co in range(2):
                    for ch in range(nch):
                        csl = slice(2 * ch, 2 * ch + 2)  # two 256-slices
                        psq = ps_proj.tile([128, 512], F32, name="psq", tag="proj")
                        for ci in range(2):
                            nc.tensor.matmul(
                                psq, wqt[:, ci, bass.ts(co, 128)],
                                xg[:, ci, csl, :].rearrange("p a b -> p (a b)"),
                                start=(ci == 0), stop=(ci == 1))
                        nc.scalar.activation(
                            out=qg[:, co, csl, :].rearrange("p a b -> p (a b)"),
                            in_=psq, func=AF.Identity, bias=bq_s[:, co:co + 1])
                        psk = ps_proj.tile([128, 512], F32, name="psk", tag="proj")
                        for ci in range(2):
                            nc.tensor.matmul(
                                psk, wkt[:, ci, bass.ts(co, 128)],
                                xg[:, ci, csl, :].rearrange("p a b -> p (a b)"),
                                start=(ci == 0), stop=(ci == 1))
                        kg_out = kg[:, co, csl, :].rearrange("p a b -> p (a b)")
                        if ch % 2 == 0:
                            nc.vector.tensor_scalar_add(
                                kg_out, psk, bk_s[:, co:co + 1])
                        else:
                            nc.scalar.activation(
                                out=kg_out, in_=psk, func=AF.Identity,
                                bias=bk_s[:, co:co + 1])

                # ---- Vt projection: [128, 2G(pos tile), 256(c)] ----
                vt = vtp.tile([128, 2 * G, C], F32R, name="vt", tag="vt")
                for pt in range(2 * G):
                    psv = ps_proj.tile([128, 512], F32, name="psv", tag="proj")
                    for ci in range(2):
                        nc.tensor.matmul(
                            psv[:, 0:C],
                            xg[:, ci, pt // 2, bass.ts(pt % 2, 128)],
                            wvt[:, ci, :],
                            start=(ci == 0), stop=(ci == 1))
                    if pt % 2 == 0:
                        nc.vector.tensor_copy(vt[:, pt, :], psv[:, 0:C])
                    else:
                        nc.scalar.copy(vt[:, pt, :], psv[:, 0:C])

                # ---- attention: process slices in pairs for N=512 tails ----
                for sp0 in range(0, G, 2):
                    # ep holds exp(St/16) for both slices: [128, 2(lt), 2(sp), 256]
                    ep = smp.tile([128, 2, 2, K], F32R, name="ep", tag="ep")
                    for sp in range(2):
                        s = sp0 + sp
                        pss = ps_attn.tile([128, 2, K], F32, name="pss", tag="at")
                        for lt in range(2):
                            for ct in range(2):
                                nc.tensor.matmul(
                                    pss[:, lt, :],
                                    kg[:, ct, s, bass.ts(lt, 128)],
                                    qg[:, ct, s, :],
                                    start=(ct == 0), stop=(ct == 1))
                        for lt in range(2):
                            nc.scalar.activation(
                                out=ep[:, lt, sp, :], in_=pss[:, lt, :],
                                func=AF.Exp, scale=SCALE)

                    # Sum over l for both slices (N=512), replicated across
                    # partitions by the ones weights; R = 1/Sum (fast approx)
                    psu = ps_attn.tile([128, 512], F32, name="psu", tag="at")
                    for lt in range(2):
                        nc.tensor.matmul(
                            psu, ones[:, 0:128],
                            ep[:, lt, :, :].rearrange("p a b -> p (a b)"),
                            start=(lt == 0), stop=(lt == 1))
                    recip = smp.tile([128, 512], F32, name="recip", tag="recip")
                    nc.vector.reciprocal_approx_fast(out=recip, in_=psu)

                    # Att = (Vt.T @ Ep) * R -> [128, 2(ct), 2(sp), 256]
                    att = smp.tile([128, 2, 2, K], F32R, name="att", tag="att")
                    for sp in range(2):
                        s = sp0 + sp
                        psa = ps_attn.tile([128, 2, K], F32, name="psa", tag="at")
                        for ct in range(2):
                            for lt in range(2):
                                nc.tensor.matmul(
                                    psa[:, ct, :],
                                    vt[:, 2 * s + lt, bass.ts(ct, 128)],
                                    ep[:, lt, sp, :],
                                    start=(lt == 0), stop=(lt == 1))
                        nc.vector.tensor_tensor(
                            out=att[:, :, sp, :], in0=psa,
                            in1=recip[:, bass.ts(sp, K)]
                                .unsqueeze(1).broadcast_to((128, 2, K)),
                            op=ALU.mult)

                    # Out = (wpT.T @ Att + bp') + x for the pair (N=512);
                    # bias+residual ride the DVE eviction (affine_then_add)
                    outf = otp.tile([128, 2, 512], F32, name="outf", tag="outf")
                    for ot in range(2):
                        psf = ps_fin.tile([128, 512], F32, name="psf", tag="fin")
                        for ct in range(2):
                            nc.tensor.matmul(
                                psf,
                                wpt[:, ct, bass.ts(ot, 128)],
                                att[:, ct, :, :].rearrange("p a b -> p (a b)"),
                                start=(ct == 0), stop=(ct == 1))
                        nc.vector.affine_then_add(
                            out=outf[:, ot, :], in0=psf,
                            in1=xg[:, ot, sp0:sp0 + 2, :]
                                .rearrange("p a b -> p (a b)"),
                            scale=1.0, bias=bpe_s[:, ot:ot + 1])
                        nc.sync.dma_start(
                            out=y_d[bass.ts(ot, 128), n0 + sp0:n0 + sp0 + 2, :],
                            in_=outf[:, ot, :].rearrange("p (a b) -> p a b", a=2))
                n0 += G

    nc.compile()
    return nc


def _get_nc():
    if "nc" not in _CACHE:
        _CACHE["nc"] = _build()
    return _CACHE["nc"]


def run(inputs, trace=False):
    x = np.ascontiguousarray(np.asarray(inputs["x"]), dtype=np.float32)
    wq = np.asarray(inputs["wq"]).astype(np.float32)
    wk = np.asarray(inputs["wk"]).astype(np.float32)
    wv = np.asarray(inputs["wv"]).astype(np.float32)
    wp = np.asarray(inputs["wp"]).astype(np.float32)
    bq = np.asarray(inputs["bq"]).astype(np.float32)
    bk = np.asarray(inputs["bk"]).astype(np.float32)
    bv = np.asarray(inputs["bv"]).astype(np.float32)
    bp = np.asarray(inputs["bp"]).astype(np.float32)

    wqt = np.ascontiguousarray(wq.T)
    wkt = np.ascontiguousarray(wk.T)
    wvt = np.ascontiguousarray(wv.T)
    wpt = np.ascontiguousarray(wp.T)
    # v-bias folds through attention (rows of prob sum to 1) into the final
    # projection bias: out = wp @ (att + bv) + bp = wp @ att + (bp + wp @ bv)
    bpe = (bp.astype(np.float64) + wp.astype(np.float64) @ bv.astype(np.float64)
           ).astype(np.float32)

    nc = _get_nc()
    common = {"wqt": wqt, "wkt": wkt, "wvt": wvt, "wpt": wpt,
              "bq": bq, "bk": bk, "bpe": bpe}
    in_maps = [dict(common, x=x[b]) for b in range(B)]
    res = run_bass_kernel_spmd(nc, in_maps, core_ids=list(range(8)), trace=trace)
    out = np.stack([res.results[b]["y"] for b in range(B)], axis=0)
    return out, res


def kernel(**inputs):
    out, _ = run(inputs, trace=False)
    return out

